# revision 1
# baseline (speedup 1.0000x reference)
"""FNO1d Trainium2 kernel (Bass/Tile), data-parallel over batch on 8 cores.

Math: with only M=16 modes kept, rfft->einsum->irfft collapses to small DFT
matmuls.  Per layer:  X = h @ F  (F [S,32] cos/sin, scaled by beta_l for fp16
range);  om = per-mode WxW complex mix;  spec = om_T @ G  (G fp32 [32,S]);
pre = spec + conv_w @ h;  h' = gelu(pre).  Final: fc1 (fp32) -> relu-trick
fc2 in fp16 (gelu(z) = relu(z) + gelu(-|z|), tail dropped: |err| <= 0.17*sum|w2|,
~9e-5 of output scale).

Layouts (per core, 4 batches as 2 pairs):
  h_nat   2 tiles [128=(b2,i), 8192] fp16     (fp32 for layer-4 output)
  hT      [128=sp, 64=c, 256=(b,i)] fp16, s = sp*64 + c, via DMA-xbar-transpose
  X~      [128=(b2,i), 96] fp16 sbuf: cols H*48 + t*16 + m, t in {Xr, Xi, -Xi}
  om      [128=(b2,o), 32] psum (col 2m+ri) -> PE-transpose -> omT [32, 256] f32
  pre     [128, 2048] psum tiles; ACT gelu drains -> next h
"""

import sys, os
for p in ("/opt/trn_rl_repo",):
    if p not in sys.path:
        sys.path.insert(0, p)

import numpy as np
import ml_dtypes
from contextlib import ExitStack

import concourse.bass as bass
import concourse.tile as tile
from concourse import bacc, mybir
from concourse.bass import _add_dep_helper

B, S, W, M, L = 32, 8192, 64, 16, 4
NCORES = 8
BPC = B // NCORES          # 4 batches per core
NPAIR = BPC // 2           # 2 pairs
FP16 = mybir.dt.float16
F32 = mybir.dt.float32
F32R = mybir.dt.float32r
AF = mybir.ActivationFunctionType

# fp16-range scales for the forward-DFT basis, per layer (X~ = X * beta)
BETA = [2.0 ** -1, 2.0 ** -3, 2.0 ** -8, 2.0 ** -13]


def build_consts(inputs):
    """Host-side constant tensors (shared by all cores)."""
    f16 = ml_dtypes.float16 if hasattr(ml_dtypes, "float16") else np.float16
    f16 = np.float16
    fc0_w = np.asarray(inputs["fc0_w"], np.float32)      # [2, W]
    fconv_wr = np.asarray(inputs["fconv_wr"], np.float32)  # [L, W, W, M]
    fconv_wi = np.asarray(inputs["fconv_wi"], np.float32)
    conv_w = np.asarray(inputs["conv_w"], np.float32)    # [L, W, W]
    fc1_w = np.asarray(inputs["fc1_w"], np.float32)      # [W, 128]
    fc2_w = np.asarray(inputs["fc2_w"], np.float32)      # [128, 1]

    s = np.arange(S, dtype=np.float64)
    m = np.arange(M, dtype=np.float64)
    ang = 2.0 * np.pi * np.outer(s, m) / S               # [S, M]
    cos = np.cos(ang)
    sin = np.sin(ang)

    # f[l]: [128, 64*32] fp16, f[l][sp, 32*c + k] = basis_k(s=sp*64+c)*beta
    f_all = np.empty((L, 128, 64 * 32), np.float16)
    basis = np.concatenate([cos, -sin], axis=1)          # [S, 32]
    # DMA-xbar transpose layout: hT[sp, c, j] = h[j, c*128 + sp]
    basis_sc = basis.reshape(64, 128, 32).transpose(1, 0, 2)   # [sp, c, k]
    for l in range(L):
        f_all[l] = (basis_sc * BETA[l]).reshape(128, 64 * 32).astype(np.float16)

    # gb: [32, S] f32, row 2m+0 = w_m*cos/S, row 2m+1 = -w_m*sin/S
    w_m = np.ones(M); w_m[1:] = 2.0
    gb = np.empty((32, S), np.float32)
    gb[0::2] = (w_m[:, None] * cos.T / S)
    gb[1::2] = (-w_m[:, None] * sin.T / S)

    # wm[l]: [128, 32*128] fp16: col-block (2m+t)*128 = blockdiag(wr/wi[:,:,m])
    wm = np.zeros((L, 128, 32 * 128), np.float16)
    for l in range(L):
        for mm in range(M):
            for t, wsrc in ((0, fconv_wr), (1, fconv_wi)):
                blk = wsrc[l, :, :, mm]                  # [i, o]
                col0 = (2 * mm + t) * 128
                wm[l, 0:64, col0:col0 + 64] = blk
                wm[l, 64:128, col0 + 64:col0 + 128] = blk

    # cw[l]: [128, 128] fp16 blockdiag of conv_w[l].T  ([i, o])
    cw = np.zeros((L, 128, 128), np.float16)
    for l in range(L):
        cw[l, 0:64, 0:64] = conv_w[l].T
        cw[l, 64:128, 64:128] = conv_w[l].T

    # fc0st: [64, 128] = 16 stacked copies of the [4, 128] block
    blk = np.zeros((4, 128), np.float32)
    blk[0, 0:64] = fc0_w[0]; blk[1, 0:64] = fc0_w[1]
    blk[2, 64:128] = fc0_w[0]; blk[3, 64:128] = fc0_w[1]
    fc0st = np.tile(blk, (16, 1))

    w1s = np.concatenate([fc1_w / 16.0, fc1_w / 16.0], axis=0).astype(np.float32)  # [128, 128] both halves
    w2s = (fc2_w * 16.0).astype(np.float16)              # [128, 1]

    # biases: [128, 8] f32: col0 fc0_b (per (b2,w)); col 1+l conv_b[l]; col5 fc1_b/16
    bias = np.zeros((128, 8), np.float32)
    fc0_b = np.asarray(inputs["fc0_b"], np.float32)
    conv_b = np.asarray(inputs["conv_b"], np.float32)
    fc1_b = np.asarray(inputs["fc1_b"], np.float32)
    bias[:, 0] = np.tile(fc0_b, 2)
    for l in range(L):
        bias[:, 1 + l] = np.tile(conv_b[l], 2)
    bias[:, 5] = fc1_b / 16.0
    ident = np.eye(128, dtype=np.float32)
    return dict(f=f_all, gb=gb, wm=wm, cw=cw, fc0st=fc0st, w1s=w1s, w2s=w2s,
                bias=bias, ident=ident)


def build_xt(x_full, core):
    """Per-core fc0 moving operand, packed for 64-partition DMA:
    xt[p, 4*q + row, col] = (x_b0, t, x_b1, t)[row] at s = q*512 + col."""
    t = np.linspace(0.0, 1.0, S, dtype=np.float32)
    xt4 = np.empty((NPAIR, 4, S), np.float32)
    for p in range(NPAIR):
        b0 = core * BPC + 2 * p
        xt4[p, 0] = x_full[b0, :, 0]
        xt4[p, 1] = t
        xt4[p, 2] = x_full[b0 + 1, :, 0]
        xt4[p, 3] = t
    return xt4


def build_program(stop=None):
    """Build + compile the per-core Bass program (identical on all cores)."""
    nc = bacc.Bacc("TRN2", target_bir_lowering=False, debug=False,
                   enable_asserts=False, num_devices=NCORES)
    dram = {}
    dram["xt"] = nc.dram_tensor("xt", [NPAIR, 4, S], F32R, kind="ExternalInput")
    dram["f"] = nc.dram_tensor("f", [L, 128, 64 * 32], FP16, kind="ExternalInput")
    dram["gb"] = nc.dram_tensor("gb", [32, S], F32R, kind="ExternalInput")
    dram["wm"] = nc.dram_tensor("wm", [L, 128, 32 * 128], FP16, kind="ExternalInput")
    dram["cw"] = nc.dram_tensor("cw", [L, 128, 128], FP16, kind="ExternalInput")
    dram["fc0st"] = nc.dram_tensor("fc0st", [64, 128], F32R, kind="ExternalInput")
    dram["w1s"] = nc.dram_tensor("w1s", [128, 128], F32R, kind="ExternalInput")
    dram["w2s"] = nc.dram_tensor("w2s", [128, 1], FP16, kind="ExternalInput")
    dram["bias"] = nc.dram_tensor("bias", [128, 8], F32, kind="ExternalInput")
    dram["ident"] = nc.dram_tensor("ident", [128, 128], F32, kind="ExternalInput")
    y_dram = nc.dram_tensor("y", [BPC, S], F32, kind="ExternalOutput")
    if stop is not None:
        dram["dbg16"] = nc.dram_tensor("dbg16", [128, S], FP16, kind="ExternalOutput")
        dram["dbg32"] = nc.dram_tensor("dbg32", [128, 512], F32, kind="ExternalOutput")
        dram["hsrc"] = nc.dram_tensor("hsrc", [128, S], FP16, kind="ExternalInput")

    with tile.TileContext(nc) as tc, ExitStack() as ctx:
        kernel_body(ctx, tc, dram, y_dram, stop)
    nc.compile()
    return nc


def kernel_body(ctx, tc, dram, y_dram, stop=None):
    nc = tc.nc
    kernel_body._last_tp = None
    def dma(out, in_, **kw):
        # The xbar (dma transpose) ucode corrupts ~1/8 of its output when
        # any plain DMA shares the sync-HWDGE queue with it.  Keep nc.sync
        # exclusively for transposes; all other DMAs go via the ACT HWDGE.
        if kw.get("transpose"):
            return nc.sync.dma_start(out, in_, **kw)
        return nc.scalar.dma_start(out, in_, **kw)
    CH = 1024                      # psum pre-tile width (fp32, 2 banks)

    pool_c = ctx.enter_context(tc.tile_pool(name="consts", bufs=1))
    pool_wm = ctx.enter_context(tc.tile_pool(name="wm", bufs=1))
    pool_f = ctx.enter_context(tc.tile_pool(name="fb", bufs=2))
    pool_h = ctx.enter_context(tc.tile_pool(name="h", bufs=6))
    pool_hT = ctx.enter_context(tc.tile_pool(name="hT", bufs=1))
    pool_sm = ctx.enter_context(tc.tile_pool(name="small", bufs=2))
    pool_ps = ctx.enter_context(tc.tile_pool(name="ps", bufs=3, space="PSUM"))
    pool_spec = ctx.enter_context(tc.tile_pool(name="spec", bufs=1, space="PSUM"))

    if stop in ("tmin", "tc1"):
        if stop == "tc1":
            gb_ = pool_c.tile([32, S], F32R, name="gb")
            dma(gb_[:], dram["gb"].ap())
            ident_ = pool_c.tile([128, 128], F32, name="ident")
            dma(ident_[:], dram["ident"].ap())
            f_0 = pool_f.tile([128, 64 * 32], FP16, tag="f", name="f_0")
            dma(f_0[:], dram["f"].ap()[0])
            wm_0 = pool_wm.tile([128, 32 * 128], FP16, tag="wm", name="wm_0")
            dma(wm_0[:], dram["wm"].ap()[0])
        h0 = pool_h.tile([128, S], FP16, tag="h", name="h0_0")
        dma(h0[:], dram["hsrc"].ap())
        hTm = pool_hT.tile([128, 64, 128], FP16, tag="hT0", name="hTmin")
        dma(hTm[:], h0[:], transpose=True)
        dma(dram["dbg16"].ap().rearrange("p (c j) -> p c j", c=64),
                          hTm[:])
        return
    # ---- constants into SBUF ----
    gb = pool_c.tile([32, S], F32R)
    dma(gb[:], dram["gb"].ap())
    fc0st = pool_c.tile([64, 128], F32R)
    dma(fc0st[:], dram["fc0st"].ap())
    w1s = pool_c.tile([128, 128], F32R)
    dma(w1s[:], dram["w1s"].ap())
    w2s = pool_c.tile([128, 1], FP16)
    dma(w2s[:], dram["w2s"].ap())
    biasT = pool_c.tile([128, 8], F32)
    dma(biasT[:], dram["bias"].ap())
    ident = pool_c.tile([128, 128], F32)
    dma(ident[:], dram["ident"].ap())
    cwT = pool_c.tile([128, L * 128], FP16)
    for l in range(L):
        dma(cwT[:, 128 * l:128 * (l + 1)], dram["cw"].ap()[l])

    # ---- fc0 ----
    h = [pool_h.tile([128, S], FP16, tag="h", name=f"h0_{p}") for p in range(NPAIR)]
    if stop in ("tc2", "tc3", "tc4"):
        for p in range(NPAIR):
            for g in range(S // CH):
                pre = pool_ps.tile([128, CH], F32, tag="ps")
                for k in range(CH // 512):
                    xt_t = pool_sm.tile([4, 512], F32R, tag="xt", name=f"xtm_{p}_{g}_{k}", bufs=3)
                    dma(xt_t[:], dram["xt"].ap()[p, :, g * CH + 512 * k:g * CH + 512 * (k + 1)])
                    if stop != "tc3":
                        nc.tensor.matmul(pre[:, 512 * k:512 * (k + 1)], lhsT=fc0st[:],
                                         rhs=xt_t[:], start=True, stop=True)
                if stop == "tc2":
                    nc.scalar.activation(h[p][:, g * CH:(g + 1) * CH], pre[:],
                                         AF.Gelu, bias=biasT[:, 0:1], scale=1.0)
                elif stop == "tc4":
                    nc.vector.tensor_copy(h[p][:, g * CH:(g + 1) * CH], pre[:])
        dma(h[0][:], dram["hsrc"].ap())
        hTm = pool_hT.tile([128, 64, 128], FP16, tag="hT0", name="hTmin")
        dma(hTm[:], h[0][:], transpose=True)
        dma(dram["dbg16"].ap().rearrange("p (c j) -> p c j", c=64),
                          hTm[:])
        return
    for p in range(NPAIR):
        for g in range(S // CH):
            pre = pool_ps.tile([128, CH], F32, tag="ps")
            xt_t = pool_sm.tile([4, CH], F32R, tag="xt",
                                name=f"xt_{p}_{g}", bufs=4)
            dma(xt_t[:], dram["xt"].ap()[p, :, g * CH:(g + 1) * CH])
            for k in range(CH // 512):
                nc.tensor.matmul(pre[:, 512 * k:512 * (k + 1)],
                                 lhsT=fc0st[0:4, :], rhs=xt_t[:, 512 * k:512 * (k + 1)],
                                 start=True, stop=True)
            nc.scalar.activation(h[p][:, g * CH:(g + 1) * CH], pre[:],
                                 AF.Gelu, bias=biasT[:, 0:1], scale=1.0)

    if stop == "fc0":
        dma(dram["dbg16"].ap(), h[0][:])
        return
    # ---- spectral layers ----
    for l in range(L):
        f_l = pool_f.tile([128, 64 * 32], FP16, tag="f")
        dma(f_l[:], dram["f"].ap()[l])
        wm_l = pool_wm.tile([128, 32 * 128], FP16, tag="wm")
        dma(wm_l[:], dram["wm"].ap()[l])

        # transpose h -> hT  (hT[sp, c, 128*p + j] = h_p[j, c*128+sp]),
        # chunked so each transpose trails its gelu chunk
        hT = pool_hT.tile([128, 64, 256], FP16, tag="hT", name=f"hT{l}")
        for g in range(4):
            for p in range(NPAIR):
                dma(hT[:, 16 * g:16 * (g + 1), 128 * p:128 * (p + 1)],
                    h[p][:, 2048 * g:2048 * (g + 1)], transpose=True)

        if stop == f"t{l}":
            dma(dram["dbg16"].ap().rearrange("p (c j) -> p c j", c=64),
                              hT[0][:])
            return
        if stop == f"tdma{l}":
            dma(dram["dbg16"].ap().rearrange("p (c j) -> p c j", c=64),
                              hT[0][:])
            return
        if stop == f"tsolo{l}":
            dma(dram["dbg16"].ap().rearrange("p (c j) -> p c j", c=64),
                              hT[0][:])
            return
        # spectral psum workspace: one CH-wide slot, col-offsets per region
        sp_ps = pool_spec.tile([128, 1024], F32, tag="spec")
        xT_ps = sp_ps[0:32, 0:256]                            # X~ [mr, (b,i)]   bank0
        xt_ps = [sp_ps[:, 256:288], sp_ps[:, 288:320]]        # X~ transposed    bank0
        om_ps = [sp_ps[:, 320:352], sp_ps[:, 512:544]]        # om H0 bank0, H1 bank1

        # DFT: X~T = sum_c F_c.T @ hT[:, c, :]   -> [32=mr, 256=(b,i)]
        for c in range(64):
            nc.tensor.matmul(xT_ps, lhsT=f_l[:, 32 * c:32 * (c + 1)],
                             rhs=hT[:, c, :], start=(c == 0), stop=(c == 63))
        # drain + PE-transpose back to [(b2,i), mr] orientation
        xT_sb = pool_sm.tile([32, 256], F32, tag="xTsb")
        nc.vector.tensor_copy(xT_sb[:], xT_ps)
        for H in range(2):
            nc.tensor.transpose(xt_ps[H], xT_sb[:, 128 * H:128 * (H + 1)], ident[0:32, 0:32])
        # X~ sbuf: [128, 128] fp16, cols 64*H + 4m + {0:Xr,1:Xi,2:-Xi,3:Xr}
        xsb = pool_sm.tile([128, 128], FP16, tag="xsb")
        for H in range(2):
            b0 = 64 * H
            nc.vector.tensor_copy(xsb[:, b0 + 0:b0 + 64:4], xt_ps[H][:, 0:16])
            nc.vector.tensor_copy(xsb[:, b0 + 3:b0 + 64:4], xt_ps[H][:, 0:16])
            nc.vector.tensor_copy(xsb[:, b0 + 1:b0 + 64:4], xt_ps[H][:, 16:32])
            nc.vector.tensor_scalar_mul(xsb[:, b0 + 2:b0 + 64:4],
                                        xt_ps[H][:, 16:32], -1.0)

        if stop == f"x{l}":
            dma(dram["dbg16"].ap()[:, 0:96], xsb[:])
            return
        # mode mix: om[(b2,o), (2m, 2m+1)] += W.T @ X~cols, N=2 per matmul
        # re = wr@xr + wi@(-xi);  im = wr@xi + wi@xr
        for mm in range(M):
            wr = wm_l[:, (2 * mm) * 128:(2 * mm + 1) * 128]
            wi = wm_l[:, (2 * mm + 1) * 128:(2 * mm + 2) * 128]
            for H in range(2):
                pair_ri = xsb[:, 64 * H + 4 * mm:64 * H + 4 * mm + 2]    # (xr, xi)
                nc.tensor.matmul(om_ps[H][:, 2 * mm:2 * mm + 2], lhsT=wr,
                                 rhs=pair_ri, start=True, stop=False,
                                 skip_group_check=True)
            for H in range(2):
                pair_nr = xsb[:, 64 * H + 4 * mm + 2:64 * H + 4 * mm + 4]  # (-xi, xr)
                nc.tensor.matmul(om_ps[H][:, 2 * mm:2 * mm + 2], lhsT=wi,
                                 rhs=pair_nr, start=False, stop=True,
                                 skip_group_check=True)

        # om -> sbuf -> PE-transpose -> omT sbuf (scaled by 1/beta)
        om_sb = pool_sm.tile([128, 64], F32, tag="omsb")
        for H in range(2):
            nc.vector.tensor_copy(om_sb[:, 32 * H:32 * (H + 1)], om_ps[H])
        omT_ps = [sp_ps[0:32, 544:672], sp_ps[0:32, 672:800]]
        omT_sb = pool_sm.tile([32, 256], F32R, tag="omT")
        for H in range(2):
            nc.tensor.transpose(omT_ps[H], om_sb[:, 32 * H:32 * (H + 1)], ident[:])
            nc.vector.tensor_scalar_mul(omT_sb[:, 128 * H:128 * (H + 1)],
                                        omT_ps[H], 1.0 / BETA[l])

        if stop == f"om{l}":
            dma(dram["dbg32"].ap()[0:32, 0:256], omT_sb[:].bitcast(F32))
            dma(dram["dbg32"].ap()[0:128, 256:256+64], om_sb[:])
            return
        # conv + spec -> pre psum; ACT gelu -> next h
        last = (l == L - 1)
        if last:
            h_next = [pool_h.tile([128, S // 2], F32R, tag="h", name=f"h4_{i}")
                      for i in range(2 * NPAIR)]     # h4: 4 tiles [128, 4096] fp32
        else:
            h_next = [pool_h.tile([128, S], FP16, tag="h", name=f"h{l+1}_{p}") for p in range(NPAIR)]
        cw_l = cwT[:, 128 * l:128 * (l + 1)]
        for p in range(NPAIR):
            for g in range(S // CH):
                pre = pool_ps.tile([128, CH], F32, tag="ps")
                for k in range(CH // 512):
                    nc.tensor.matmul(pre[:, 512 * k:512 * (k + 1)], lhsT=cw_l,
                                     rhs=h[p][:, g * CH + 512 * k:g * CH + 512 * (k + 1)],
                                     start=True, stop=False, skip_group_check=True)
                for k in range(CH // 512):
                    nc.tensor.matmul(
                        pre[:, 512 * k:512 * (k + 1)],
                        lhsT=omT_sb[:, 128 * p:128 * (p + 1)],
                        rhs=gb[:, g * CH + 512 * k:g * CH + 512 * (k + 1)],
                        start=False, stop=True, skip_group_check=True)
                if last:
                    dst = h_next[2 * p + g // (4096 // CH)][:, (g % (4096 // CH)) * CH:(g % (4096 // CH) + 1) * CH]
                else:
                    dst = h_next[p][:, g * CH:(g + 1) * CH]
                nc.scalar.activation(dst, pre[:], AF.Gelu,
                                     bias=biasT[:, 1 + l:2 + l], scale=1.0)
        h = h_next
        if stop == f"layer{l}":
            if l < L - 1:
                dma(dram["dbg16"].ap(), h[0][:])
            else:
                dma(dram["dbg32"].ap()[:, 0:512], h[0][:, 0:512].bitcast(F32))
            return

    # ---- fc1 (fp32, w1/16) + relu -> g~ fp16; fc2 via g~-chunk stationaries ----
    h4 = h                                  # 4 tiles [128, 4096] f32r: (pair, s-half)
    y_sb = pool_sm.tile([128, 256], F32, tag="ysb")
    for p in range(NPAIR):
        for b2 in range(2):
            for sh in range(2):
                b = 2 * p + b2
                gt = pool_h.tile([128, S // 2], FP16, tag="h",
                                 name=f"gt_{b}_{sh}")
                for g in range(4096 // CH):
                    pre = pool_ps.tile([128, CH], F32, tag="ps")
                    for k in range(CH // 512):
                        nc.tensor.matmul(
                            pre[:, 512 * k:512 * (k + 1)],
                            lhsT=w1s[64 * b2:64 * (b2 + 1), :],
                            rhs=h4[2 * p + sh][64 * b2:64 * (b2 + 1),
                                               g * CH + 512 * k:g * CH + 512 * (k + 1)],
                            start=True, stop=True)
                    nc.scalar.activation(gt[:, g * CH:(g + 1) * CH], pre[:],
                                         AF.Relu, bias=biasT[:, 5:6], scale=1.0)
                y_ps = pool_ps.tile([128, CH], F32, tag="ps",
                                    name=f"yps_{b}_{sh}")
                for k in range(32):
                    nc.tensor.matmul(y_ps[:, k:k + 1],
                                     lhsT=gt[:, 128 * k:128 * (k + 1)], rhs=w2s[:],
                                     start=True, stop=True)
                nc.vector.tensor_copy(y_sb[:, 64 * b + 32 * sh:64 * b + 32 * sh + 32],
                                      y_ps[:, 0:32])
    dma(
        y_dram.ap().rearrange("b (sc sp) -> sp b sc", sp=128),
        y_sb[:].rearrange("sp (b sc) -> sp b sc", b=BPC))


_PROGRAM = None


def _get_program():
    global _PROGRAM
    if _PROGRAM is None:
        _PROGRAM = build_program()
    return _PROGRAM


def kernel(**inputs):
    from concourse.bass_utils import run_bass_kernel_spmd
    nc = _get_program()
    consts = build_consts(inputs)
    x_full = np.asarray(inputs["x"], np.float32)
    in_maps = []
    for core in range(NCORES):
        im = {k: v for k, v in consts.items()}
        im["xt"] = build_xt(x_full, core)
        in_maps.append(im)
    res = run_bass_kernel_spmd(nc, in_maps, list(range(NCORES)))
    y = np.concatenate([res.results[i]["y"] for i in range(NCORES)], axis=0)
    y = y + np.asarray(inputs["fc2_b"], np.float32)[0]
    return y.reshape(B, S, 1).astype(np.float32)

